# revision 17
# baseline (speedup 1.0000x reference)
"""MultiHeadGAT layer on 8 TRN2 cores.

Strategy (graph-parallel, per-core full table):
- Host packs nodes into 400 windows of <=128 nodes (greedy balance on
  in-degree, 50 windows per core = 25 window-pairs). Each core gets its own
  node permutation with its 50 destination windows first.
- Stage 1 (per core): full per-node table row
  [xh(256) | s_src(4) | s_dst(4) | pad] via one 288-col matmul per window
  (4 windows per PSUM tile, one cast-copy + one strided DMA per 4 windows;
  rows strided at 768B in DRAM, only 576B written). s_dst for the core's own
  50 windows is extracted into SBUF.
- Stage 2 (per core, per window-PAIR): two dma_gathers (lo/hi int16 tables)
  fetch source rows for 2*K chunks of 128 edge slots; host-precomputed fp8
  one-hot matrices (slot-major for the segment matmul, dst-major for the
  s_dst expansion) are DMA-loaded; attention weights are computed per slot;
  per-window segment matmuls accumulate numerator+denominator in PSUM;
  normalize, project with bf16 transposes + node-major output matmuls,
  ELU + residual + LayerNorm (bn_stats), write 256 rows per pair.
- Host scatters the 8 per-core outputs back to original node order.
"""

import math
import heapq
import numpy as np

import ml_dtypes
import concourse.bacc as bacc
import concourse.bass as bass
import concourse.tile as tile
from concourse import mybir
from concourse.bass_utils import run_bass_kernel_spmd

F32 = mybir.dt.float32
BF16 = mybir.dt.bfloat16
FP8 = mybir.dt.float8e4
NPBF = ml_dtypes.bfloat16
NPF8 = ml_dtypes.float8_e4m3
I16 = mybir.dt.int16
OH = mybir.dt.bfloat16
NPOH = NPBF
AX = mybir.AxisListType.X
OP = mybir.AluOpType
ACT = mybir.ActivationFunctionType

N, D, H, E = 50000, 64, 4, 400000
NCORES = 8
WPC = 50                 # windows per core
NP = WPC // 2            # 25 window pairs per core
WG = NCORES * WPC        # 400 global windows
ROWS = WG * 128          # 51200 table rows
LO = 32768               # lo-table rows; hi-table = ROWS - LO
RC = 384                 # table row stride in bf16 elements (768B)
WCOL = 288               # columns actually written (576B, 64B-aligned)
C = 260                  # used columns of a table row (xh + s_src)
RW = 264                 # matmul content cols: xh + s_src + s_dst
PAD_DST = 999.0


def preprocess(x, edge_index, W_lin, attn_src, attn_dst, W_out, b_out, ln_g, ln_b):
    """Returns (in_maps, (K_lo, K_hi, flags), scatter_info)."""
    x = np.asarray(x, np.float32)
    ei = np.asarray(edge_index)
    dst = ei[0].astype(np.int64)
    src = ei[1].astype(np.int64)
    W_lin = np.asarray(W_lin, np.float32)
    attn_src = np.asarray(attn_src, np.float32)
    attn_dst = np.asarray(attn_dst, np.float32)
    W_out = np.asarray(W_out, np.float32)
    b_out = np.asarray(b_out, np.float32)
    ln_g = np.asarray(ln_g, np.float32)
    ln_b = np.asarray(ln_b, np.float32)

    deg = np.bincount(dst, minlength=N)

    # --- pack nodes into WG windows: <=128 nodes each, balanced edge sums ---
    order = np.argsort(-deg, kind="stable")
    heap = [(0, w) for w in range(WG)]
    heapq.heapify(heap)
    win_nodes = [[] for _ in range(WG)]
    win_sum = [0] * WG
    for v in order:
        s, w = heapq.heappop(heap)
        win_nodes[w].append(v)
        win_sum[w] = s + int(deg[v])
        if len(win_nodes[w]) < 128:
            heapq.heappush(heap, (win_sum[w], w))

    slot_nodes = np.zeros((WG, 128), np.int64)
    slot_valid = np.zeros((WG, 128), bool)
    for w in range(WG):
        n = len(win_nodes[w])
        slot_nodes[w, :n] = win_nodes[w]
        slot_valid[w, :n] = True

    window_of = np.empty(N, np.int64)
    pos_in_window = np.empty(N, np.int64)
    window_of[slot_nodes[slot_valid]] = np.nonzero(slot_valid)[0]
    pos_in_window[slot_nodes[slot_valid]] = np.nonzero(slot_valid)[1]

    core_of_edge = window_of[dst] // WPC

    # consts shared by all cores
    ident = np.eye(128, dtype=np.float32).astype(NPBF)
    v_src = np.stack([W_lin[h * D:(h + 1) * D, :].T @ attn_src[h] for h in range(H)], axis=1)
    v_dst = np.stack([W_lin[h * D:(h + 1) * D, :].T @ attn_dst[h] for h in range(H)], axis=1)
    rhs = np.concatenate(
        [W_lin.T, v_src, v_dst, np.zeros((D, WCOL - RW), np.float32)], axis=1
    ).astype(NPBF)                                    # [64, 288]
    woutb = np.ascontiguousarray(W_out.T).astype(NPBF)  # [256, 64]
    bout_row = b_out.reshape(1, D).astype(NPBF)       # [1, 64]
    ones_row = np.ones((1, 128), NPBF)
    lng = np.tile(ln_g.reshape(1, 1, D), (128, 2, 1)).astype(np.float32).reshape(128, 2 * D)
    lnb = np.tile(ln_b.reshape(1, 1, D), (128, 2, 1)).astype(np.float32).reshape(128, 2 * D)

    # first pass per core: per-window lo/hi edge counts to size K_lo/K_hi
    per_core = []
    max_lo = max_hi = 0
    for c in range(NCORES):
        own = np.arange(c * WPC, (c + 1) * WPC)
        others = np.concatenate([np.arange(0, c * WPC), np.arange((c + 1) * WPC, WG)])
        worder = np.concatenate([own, others])
        perm = slot_nodes[worder].reshape(-1)
        val = slot_valid[worder].reshape(-1)
        row_of = np.empty(N, np.int64)
        row_of[perm[val]] = np.nonzero(val)[0]

        eidx = np.nonzero(core_of_edge == c)[0]
        wl = (window_of[dst[eidx]] - c * WPC).astype(np.int64)
        srow = row_of[src[eidx]]
        islo = srow < LO
        nlo = np.bincount(wl[islo], minlength=WPC)
        nhi = np.bincount(wl[~islo], minlength=WPC)
        max_lo = max(max_lo, int(nlo.max()))
        max_hi = max(max_hi, int(nhi.max()))
        per_core.append((perm, row_of, eidx, wl, srow, islo, nlo, nhi))

    K_lo = math.ceil(max_lo / 128)
    K_hi = math.ceil(max_hi / 128)
    K = K_lo + K_hi
    K2 = 2 * K
    cnt_lo = np.max(np.stack([pc[6] for pc in per_core]), axis=0)  # [WPC]
    cnt_hi = np.max(np.stack([pc[7] for pc in per_core]), axis=0)

    in_maps = []
    for c in range(NCORES):
        perm, row_of, eidx, wl, srow, islo, _, _ = per_core[c]
        xTp = np.ascontiguousarray(x[perm].T).astype(NPBF)  # [64, ROWS]
        xres = np.ascontiguousarray(
            (x[perm[:WPC * 128]] - 1.0).reshape(WPC, 128, D).transpose(1, 0, 2).reshape(128, WPC * D))

        # per-pair slot assignment; pair chunk layout:
        #   [w0-lo(K_lo) | w1-lo(K_lo) | w0-hi(K_hi) | w1-hi(K_hi)]
        idx_lo = np.zeros((NP, 2 * K_lo * 128), np.int16)
        idx_hi = np.zeros((NP, 2 * K_hi * 128), np.int16)
        dstloc = np.full((NP, 128, K2), PAD_DST, np.float32)
        for mask, base, ishi in [(islo, 0, 0), (~islo, LO, 1)]:
            sel = np.nonzero(mask)[0]
            wls = wl[sel]
            o2 = np.argsort(wls, kind="stable")
            sel = sel[o2]
            wls = wls[o2]
            counts = np.bincount(wls, minlength=WPC)
            starts = np.concatenate([[0], np.cumsum(counts)[:-1]])
            s = np.arange(len(sel)) - starts[wls]       # slot within window region
            pr = wls // 2                               # pair
            j = wls % 2                                 # window within pair
            Kr = K_hi if ishi else K_lo
            spos = j * Kr * 128 + s                     # position in region stream
            if ishi:
                idx_hi[pr, spos] = (srow[sel] - base).astype(np.int16)
            else:
                idx_lo[pr, spos] = (srow[sel] - base).astype(np.int16)
            ch = (2 * K_lo if ishi else 0) + j * Kr + s // 128  # pair chunk
            dstloc[pr, s % 128, ch] = pos_in_window[dst[eidx[sel]]].astype(np.float32)

        # wrap int16 indices: [128, L//16] (16-partition wrap replicated x8)
        idx16 = np.zeros((128, NP * K2 * 8), np.int16)
        for pr in range(NP):
            colbase = pr * K2 * 8
            blk_lo = idx_lo[pr].reshape(2 * K_lo * 8, 16).T
            idx16[:, colbase:colbase + 2 * K_lo * 8] = np.tile(blk_lo, (8, 1))
            if K_hi:
                blk_hi = idx_hi[pr].reshape(2 * K_hi * 8, 16).T
                idx16[:, colbase + 2 * K_lo * 8:colbase + K2 * 8] = np.tile(blk_hi, (8, 1))

        # one-hot matrices, fp8: st (slot-major) and snm (dst-major)
        r128 = np.arange(128, dtype=np.float32)
        st3 = (dstloc[:, :, :, None] == r128).astype(NPOH)       # [NP,128,K2,128]
        snm3 = np.ascontiguousarray(st3.transpose(0, 3, 2, 1))   # [NP,128,K2,128]
        oh = np.concatenate(
            [st3.reshape(NP, 128, K2 * 128), snm3.reshape(NP, 128, K2 * 128)], axis=2
        ).transpose(1, 0, 2).reshape(128, NP * 2 * K2 * 128)
        oh = np.ascontiguousarray(oh)

        in_maps.append({
            "xTp": xTp, "xres": xres, "idx16": idx16, "oh": oh,
            "ident": ident, "rhs": rhs, "woutb": woutb,
            "bout_row": bout_row, "ones_row": ones_row,
            "lng": lng, "lnb": lnb,
            "epsc": np.full((128, 1), 1e-5, np.float32),
        })

    flags = {
        "skip_bout": bool(np.all(b_out == 0.0)),
        "skip_ln_affine": bool(np.all(ln_g == 1.0) and np.all(ln_b == 0.0)),
        "cnt_lo": [int(v) for v in cnt_lo],
        "cnt_hi": [int(v) for v in cnt_hi],
    }
    scatter = (slot_nodes, slot_valid)
    return in_maps, (K_lo, K_hi, flags), scatter


def postprocess(results, scatter):
    slot_nodes, slot_valid = scatter
    y = np.empty((N, D), np.float32)
    for c in range(NCORES):
        oc = results[c]["y"]
        own = np.arange(c * WPC, (c + 1) * WPC)
        nodes = slot_nodes[own].reshape(-1)
        val = slot_valid[own].reshape(-1)
        y[nodes[val]] = oc[val]
    return y


def _filter_act_tables():
    """Keep only natural_log_exp_and_others as a loadable ACT set (indices
    preserved) so every activation in the kernel shares one table load."""
    import concourse.hw_specs as hw_specs
    if getattr(hw_specs, "_gat_patched", False):
        return
    orig = hw_specs.get_activation_tables

    def patched(module_arch):
        tabs = orig(module_arch)
        keep = "natural_log_exp_and_others"
        if keep in tabs:
            tabs = {k: (v if k == keep else set()) for k, v in tabs.items()}
        return tabs

    hw_specs.get_activation_tables = patched
    try:
        import concourse.bacc as _bacc_mod
        if getattr(_bacc_mod, "get_activation_tables", None) is orig:
            _bacc_mod.get_activation_tables = patched
    except Exception:
        pass
    hw_specs._gat_patched = True


def build_nc(K_lo, K_hi, flags=None, num_devices=NCORES, debug=False):
    flags = flags or {}
    _filter_act_tables()
    K = K_lo + K_hi
    K2 = 2 * K

    def chunk_owner(ch):
        return ch // K_lo if ch < 2 * K_lo else (ch - 2 * K_lo) // K_hi

    nc = bacc.Bacc("TRN2", target_bir_lowering=False, debug=False,
                   num_devices=num_devices, num_swdge_queues=4)
    xTp_d = nc.dram_tensor("xTp", [D, ROWS], BF16, kind="ExternalInput")
    xres_d = nc.dram_tensor("xres", [128, WPC * D], F32, kind="ExternalInput")
    idx16_d = nc.dram_tensor("idx16", [128, NP * K2 * 8], I16, kind="ExternalInput")
    oh_d = nc.dram_tensor("oh", [128, NP * 2 * K2 * 128], OH, kind="ExternalInput")
    ident_d = nc.dram_tensor("ident", [128, 128], BF16, kind="ExternalInput")
    rhs_d = nc.dram_tensor("rhs", [D, WCOL], BF16, kind="ExternalInput")
    woutb_d = nc.dram_tensor("woutb", [H * D, D], BF16, kind="ExternalInput")
    bout_d = nc.dram_tensor("bout_row", [1, D], BF16, kind="ExternalInput")
    ones_d = nc.dram_tensor("ones_row", [1, 128], BF16, kind="ExternalInput")
    lng_d = nc.dram_tensor("lng", [128, 2 * D], F32, kind="ExternalInput")
    lnb_d = nc.dram_tensor("lnb", [128, 2 * D], F32, kind="ExternalInput")
    epsc_d = nc.dram_tensor("epsc", [128, 1], F32, kind="ExternalInput")
    y_d = nc.dram_tensor("y", [WPC * 128, D], F32, kind="ExternalOutput")
    table_lo = nc.dram_tensor("table_lo", [LO, RC], BF16)
    table_hi = nc.dram_tensor("table_hi", [ROWS - LO, RC], BF16)

    with tile.TileContext(nc) as tc:
        with tc.tile_pool(name="const", bufs=1) as cp:
            ident = cp.tile([128, 128], BF16); nc.sync.dma_start(out=ident[:], in_=ident_d[:])
            rhs = cp.tile([D, WCOL], BF16); nc.sync.dma_start(out=rhs[:], in_=rhs_d[:])
            wout0 = cp.tile([128, D], BF16); nc.sync.dma_start(out=wout0[:], in_=woutb_d[0:128, :])
            wout1 = cp.tile([128, D], BF16); nc.sync.dma_start(out=wout1[:], in_=woutb_d[128:256, :])
            boutr = cp.tile([1, D], BF16); nc.sync.dma_start(out=boutr[:], in_=bout_d[:])
            onesr = cp.tile([1, 128], BF16); nc.sync.dma_start(out=onesr[:], in_=ones_d[:])
            lng = cp.tile([128, 2 * D], F32); nc.sync.dma_start(out=lng[:], in_=lng_d[:])
            lnb = cp.tile([128, 2 * D], F32); nc.sync.dma_start(out=lnb[:], in_=lnb_d[:])
            epsc = cp.tile([128, 1], F32); nc.sync.dma_start(out=epsc[:], in_=epsc_d[:])
            xres = cp.tile([128, WPC * D], F32); nc.sync.dma_start(out=xres[:], in_=xres_d[:])
            idx16 = cp.tile([128, NP * K2 * 8], I16); nc.sync.dma_start(out=idx16[:], in_=idx16_d[:])
            sdstb = cp.tile([128, WPC * H], BF16)

            # ---- stage 1: build table, 4 windows per PSUM tile/copy/DMA ----
            XCH = 16
            WB = 4
            with tc.tile_pool(name="s1x", bufs=3) as s1x, \
                 tc.tile_pool(name="s1row", bufs=3) as s1row, \
                 tc.tile_pool(name="s1p", bufs=2, space="PSUM") as s1p:
                for wb in range(0, WG, XCH):
                    xt = s1x.tile([D, XCH * 128], BF16, tag="xt")
                    nc.sync.dma_start(out=xt[:], in_=xTp_d[:, wb * 128:(wb + XCH) * 128])
                    for g4 in range(0, XCH, WB):
                        w0 = wb + g4
                        ps = s1p.tile([128, WB, 512], F32, tag="ps")
                        for j in range(WB):
                            nc.tensor.matmul(ps[:, j, 0:WCOL],
                                             lhsT=xt[:, (g4 + j) * 128:(g4 + j + 1) * 128],
                                             rhs=rhs[:], start=True, stop=True)
                        row4 = s1row.tile([128, WB, WCOL], BF16, tag="row")
                        if (w0 // WB) % 2 == 0:
                            nc.scalar.activation(row4[:], ps[:, :, 0:WCOL], ACT.Copy)
                        else:
                            nc.vector.tensor_copy(row4[:], ps[:, :, 0:WCOL])
                        n_own = max(0, min(WB, WPC - w0))
                        if n_own > 0:
                            nc.vector.tensor_copy(
                                sdstb[:, w0 * H:(w0 + n_own) * H].rearrange(
                                    "p (t f) -> p t f", f=H),
                                row4[:, 0:n_own, 260:264])
                        r0 = w0 * 128
                        dt_ = table_lo if r0 < LO else table_hi
                        base = r0 if r0 < LO else r0 - LO
                        nc.sync.dma_start(
                            out=dt_[base:base + WB * 128, 0:WCOL].rearrange(
                                "(t p) f -> p t f", p=128),
                            in_=row4[:])

            # ---- stage 2: per window-pair message passing ----
            with tc.tile_pool(name="gat", bufs=3) as gat, \
                 tc.tile_pool(name="ohp", bufs=4) as ohp, \
                 tc.tile_pool(name="mp", bufs=2) as mpp, \
                 tc.tile_pool(name="aop", bufs=2) as aop, \
                 tc.tile_pool(name="atp", bufs=2) as atp, \
                 tc.tile_pool(name="sm", bufs=8) as sm, \
                 tc.tile_pool(name="pSd", bufs=2, space="PSUM") as pSd, \
                 tc.tile_pool(name="pSeg", bufs=1, space="PSUM") as pSeg, \
                 tc.tile_pool(name="pT", bufs=2, space="PSUM") as pT, \
                 tc.tile_pool(name="pP", bufs=2, space="PSUM") as pP:

                g_t = [None] * NP
                oh_t = [None] * NP
                sd_t = [None] * NP
                aex_t = [None] * NP

                cnt_lo = flags.get("cnt_lo") or [K_lo * 128] * WPC
                cnt_hi = flags.get("cnt_hi") or [K_hi * 128] * WPC

                def prep(p):
                    # 4 gathers per pair (one per window per region): each must
                    # stay under the 1024-descriptor SWDGE ring carveout.
                    g = gat.tile([128, K2 * RC], BF16, tag="g")
                    off = p * K2 * 8
                    for j in range(2):
                        nc.gpsimd.dma_gather(
                            out_ap=g[:, j * K_lo * RC:(j + 1) * K_lo * RC]
                                .rearrange("p (k e) -> p k e", e=RC),
                            in_ap=table_lo[:],
                            idxs_ap=idx16[:, off + j * K_lo * 8:off + (j + 1) * K_lo * 8],
                            num_idxs=K_lo * 128, num_idxs_reg=K_lo * 128,
                            elem_size=RC, queue_num=(4 * p + j) % 4)
                        nc.gpsimd.dma_gather(
                            out_ap=g[:, (2 * K_lo + j * K_hi) * RC:
                                     (2 * K_lo + (j + 1) * K_hi) * RC]
                                .rearrange("p (k e) -> p k e", e=RC),
                            in_ap=table_hi[:],
                            idxs_ap=idx16[:, off + 2 * K_lo * 8 + j * K_hi * 8:
                                          off + 2 * K_lo * 8 + (j + 1) * K_hi * 8],
                            num_idxs=K_hi * 128, num_idxs_reg=K_hi * 128,
                            elem_size=RC, queue_num=(4 * p + 2 + j) % 4)
                    g_t[p] = g
                    oh = ohp.tile([128, 2 * K2 * 128], OH, tag="oh")
                    nc.scalar.dma_start(
                        out=oh[:], in_=oh_d[:, p * 2 * K2 * 128:(p + 1) * 2 * K2 * 128])
                    oh_t[p] = oh

                def alpha(p):
                    # s_dst per slot via dst-major one-hot matmuls, then
                    # alpha_exp into SBUF (written later into mval cols 256:260)
                    g, oh = g_t[p], oh_t[p]
                    g3 = g[:].rearrange("p (k f) -> p k f", f=RC)
                    sd = pSd.tile([128, K2 * H], F32, tag="sd")
                    for ch in range(K2):
                        nc.tensor.matmul(
                            sd[:, ch * H:(ch + 1) * H],
                            lhsT=oh[:, (K2 + ch) * 128:(K2 + ch + 1) * 128],
                            rhs=sdstb[:, (2 * p + chunk_owner(ch)) * H:
                                      (2 * p + chunk_owner(ch) + 1) * H],
                            start=True, stop=True)
                    apre = sm.tile([128, K2 * H], F32, tag="apre")
                    nc.vector.tensor_tensor(
                        out=apre[:].rearrange("p (k h) -> p k h", h=H),
                        in0=g3[:, :, 256:260],
                        in1=sd[:].rearrange("p (k h) -> p k h", h=H),
                        op=OP.add)
                    lr = sm.tile([128, K2 * H], F32, tag="lr")
                    nc.scalar.activation(lr[:], apre[:], ACT.Prelu, alpha=0.2)
                    aex = sm.tile([128, K2 * H], BF16, tag="aex")
                    nc.scalar.activation(aex[:], lr[:], ACT.Exp)
                    sd_t[p] = sd
                    aex_t[p] = aex

                def tail(p):
                    g, oh, aex = g_t[p], oh_t[p], aex_t[p]
                    g3 = g[:].rearrange("p (k f) -> p k f", f=RC)
                    a3 = aex[:].rearrange("p (k h) -> p k h", h=H)

                    # weighted messages [128, K2, 260]
                    mval = mpp.tile([128, K2 * C], BF16, tag="m")
                    m3 = mval[:].rearrange("p (k f) -> p k f", f=C)
                    nc.scalar.activation(m3[:, :, 256:260], a3, ACT.Copy)
                    nc.vector.tensor_tensor(
                        out=m3[:, :, 0:256].rearrange("p k (h d) -> p k h d", d=D),
                        in0=g3[:, :, 0:256].rearrange("p k (h d) -> p k h d", d=D),
                        in1=a3.unsqueeze(-1).to_broadcast([128, K2, H, D]),
                        op=OP.mult)

                    # segment matmuls: per window j, over its K chunks
                    seg = pSeg.tile([128, 2, 512], F32, tag="seg")
                    for j in range(2):
                        chunks = ([j * K_lo + i for i in range(K_lo)] +
                                  [2 * K_lo + j * K_hi + i for i in range(K_hi)])
                        for ki, ch in enumerate(chunks):
                            nc.tensor.matmul(
                                seg[:, j, 0:C],
                                lhsT=oh[:, ch * 128:(ch + 1) * 128],
                                rhs=mval[:, ch * C:(ch + 1) * C],
                                start=(ki == 0), stop=(ki == K - 1))

                    # normalize by denominators -> ao bf16 [128, 2, 256]
                    d1 = sm.tile([128, 2 * H], F32, tag="d1")
                    nc.vector.tensor_scalar_add(
                        d1[:].rearrange("p (t h) -> p t h", h=H),
                        seg[:, :, 256:260], 1e-9)
                    rec = sm.tile([128, 2 * H], F32, tag="rec")
                    nc.vector.reciprocal(rec[:], d1[:])
                    ao = aop.tile([128, 2, 256], BF16, tag="ao")
                    nc.vector.tensor_tensor(
                        out=ao[:].rearrange("p t (h d) -> p t h d", d=D),
                        in0=seg[:, :, 0:256].rearrange("p t (h d) -> p t h d", d=D),
                        in1=rec[:].rearrange("p (t h) -> p t h", h=H)
                            .unsqueeze(-1).to_broadcast([128, 2, H, D]),
                        op=OP.mult)

                    # transpose ao -> aTs [128, 4, 128] (hd-major chunks)
                    tp = pT.tile([128, 4, 128], BF16, tag="tp")
                    aof = ao[:].rearrange("p t f -> p (t f)")
                    for i in range(4):
                        nc.tensor.transpose(tp[:, i, :], aof[:, i * 128:(i + 1) * 128],
                                            ident[:])
                    aTs = atp.tile([128, 4, 128], BF16, tag="aTs")
                    nc.scalar.activation(aTs[:], tp[:], ACT.Copy)

                    # project node-major: prj[node, d] = sum_hd aT[hd,node]*woutT[hd,d]
                    prj = pP.tile([128, 2, D], F32, tag="prj")
                    for j in range(2):
                        nc.tensor.matmul(prj[:, j, :], lhsT=aTs[:, 2 * j, :],
                                         rhs=wout0[:], start=True, stop=False)
                        nc.tensor.matmul(prj[:, j, :], lhsT=aTs[:, 2 * j + 1, :],
                                         rhs=wout1[:],
                                         start=False, stop=flags.get("skip_bout", False))
                        if not flags.get("skip_bout"):
                            nc.tensor.matmul(prj[:, j, :], lhsT=onesr[:],
                                             rhs=boutr[:], start=False, stop=True)

                    # ELU + residual(x-1): y2 = max(o,0) + exp(min(o,0)) + (x-1)
                    mn = sm.tile([128, 2 * D], F32, tag="mn")
                    nc.vector.tensor_scalar_min(
                        mn[:].rearrange("p (t f) -> p t f", f=D), prj[:], 0.0)
                    ex = sm.tile([128, 2 * D], F32, tag="ex")
                    nc.scalar.activation(ex[:], mn[:], ACT.Exp)
                    px = sm.tile([128, 2 * D], F32, tag="px")
                    nc.vector.tensor_scalar_max(
                        px[:].rearrange("p (t f) -> p t f", f=D), prj[:], 0.0)
                    y1 = sm.tile([128, 2 * D], F32, tag="y1")
                    nc.vector.tensor_tensor(out=y1[:], in0=px[:], in1=ex[:], op=OP.add)
                    y2 = sm.tile([128, 2 * D], F32, tag="y2")
                    nc.vector.tensor_tensor(
                        out=y2[:], in0=y1[:],
                        in1=xres[:, 2 * p * D:(2 * p + 2) * D], op=OP.add)

                    # LayerNorm via bn_stats; rstd = exp(-0.5 ln(var + eps))
                    y23 = y2[:].rearrange("p (t f) -> p t f", f=D)
                    stats = sm.tile([128, 2, 6], F32, tag="stats")
                    nc.vector.bn_stats(out=stats[:, 0, :], in_=y23[:, 0, :])
                    nc.vector.bn_stats(out=stats[:, 1, :], in_=y23[:, 1, :])
                    mv = sm.tile([128, 2, 2], F32, tag="mv")
                    nc.vector.bn_aggr(out=mv[:, 0, :], in_=stats[:, 0, :])
                    nc.vector.bn_aggr(out=mv[:, 1, :], in_=stats[:, 1, :])
                    lnv = sm.tile([128, 2], F32, tag="lnv")
                    nc.scalar.activation(lnv[:], mv[:, :, 1], ACT.Ln, bias=epsc[:, 0:1])
                    rstd = sm.tile([128, 2], F32, tag="rstd")
                    nc.scalar.activation(rstd[:], lnv[:], ACT.Exp, scale=-0.5)
                    cen = sm.tile([128, 2 * D], F32, tag="cen")
                    nc.vector.tensor_tensor(
                        out=cen[:].rearrange("p (t f) -> p t f", f=D),
                        in0=y23,
                        in1=mv[:, :, 0:1].to_broadcast([128, 2, D]),
                        op=OP.subtract)
                    f1 = sm.tile([128, 2 * D], F32, tag="f1")
                    nc.vector.tensor_tensor(
                        out=f1[:].rearrange("p (t f) -> p t f", f=D),
                        in0=cen[:].rearrange("p (t f) -> p t f", f=D),
                        in1=rstd[:].unsqueeze(-1).to_broadcast([128, 2, D]),
                        op=OP.mult)
                    if not flags.get("skip_ln_affine"):
                        f2 = sm.tile([128, 2 * D], F32, tag="f2")
                        nc.vector.tensor_tensor(out=f2[:], in0=f1[:], in1=lng[:], op=OP.mult)
                        f3 = sm.tile([128, 2 * D], F32, tag="f3")
                        nc.vector.tensor_tensor(out=f3[:], in0=f2[:], in1=lnb[:], op=OP.add)
                        f1 = f3
                    nc.sync.dma_start(
                        out=y_d[p * 256:(p + 1) * 256, :].rearrange("(t p) f -> p t f", p=128),
                        in_=f1[:].rearrange("p (t f) -> p t f", f=D))
                    g_t[p] = oh_t[p] = sd_t[p] = aex_t[p] = None

                for p0 in range(min(3, NP)):
                    prep(p0)
                alpha(0)
                for p in range(NP):
                    if p + 1 < NP:
                        alpha(p + 1)
                    tail(p)
                    if p + 3 < NP:
                        prep(p + 3)

    nc.finalize()
    return nc


def run(inputs, trace=False, num_devices=NCORES):
    in_maps, (K_lo, K_hi, flags), scatter = preprocess(**inputs)
    print("K_lo, K_hi, flags:", K_lo, K_hi, flags)
    nc = build_nc(K_lo, K_hi, flags, num_devices=num_devices)
    res = run_bass_kernel_spmd(nc, in_maps, core_ids=list(range(num_devices)), trace=trace)
    y = postprocess(res.results, scatter)
    return y, res


def kernel(**inputs):
    """Full-input MultiHeadGAT layer on 8 TRN2 NeuronCores."""
    y, _ = run(inputs, trace=False)
    return y


# revision 18
# speedup vs baseline: 1.2999x; 1.2999x over previous
"""MultiHeadGAT layer on 8 TRN2 cores.

Strategy (graph-parallel, per-core full table):
- Host packs nodes into 400 windows of <=128 nodes (greedy balance on
  in-degree, 50 windows per core = 25 window-pairs). Each core gets its own
  node permutation with its 50 destination windows first.
- Stage 1 (per core): full per-node table row
  [xh(256) | s_src(4) | s_dst(4) | pad] via one 288-col matmul per window
  (4 windows per PSUM tile, one cast-copy + one strided DMA per 4 windows;
  rows strided at 768B in DRAM, only 576B written). s_dst for the core's own
  50 windows is extracted into SBUF.
- Stage 2 (per core, per window-PAIR): two dma_gathers (lo/hi int16 tables)
  fetch source rows for 2*K chunks of 128 edge slots; host-precomputed fp8
  one-hot matrices (slot-major for the segment matmul, dst-major for the
  s_dst expansion) are DMA-loaded; attention weights are computed per slot;
  per-window segment matmuls accumulate numerator+denominator in PSUM;
  normalize, project with bf16 transposes + node-major output matmuls,
  ELU + residual + LayerNorm (bn_stats), write 256 rows per pair.
- Host scatters the 8 per-core outputs back to original node order.
"""

import math
import heapq
import numpy as np

import ml_dtypes
import concourse.bacc as bacc
import concourse.bass as bass
import concourse.tile as tile
from concourse import mybir
from concourse.bass_utils import run_bass_kernel_spmd

F32 = mybir.dt.float32
BF16 = mybir.dt.bfloat16
FP8 = mybir.dt.float8e4
NPBF = ml_dtypes.bfloat16
NPF8 = ml_dtypes.float8_e4m3
I16 = mybir.dt.int16
OH = mybir.dt.bfloat16
NPOH = NPBF
AX = mybir.AxisListType.X
OP = mybir.AluOpType
ACT = mybir.ActivationFunctionType

N, D, H, E = 50000, 64, 4, 400000
NCORES = 8
WPC = 50                 # windows per core
NP = WPC // 2            # 25 window pairs per core
WG = NCORES * WPC        # 400 global windows
ROWS = WG * 128          # 51200 table rows
LO = 32768               # lo-table rows; hi-table = ROWS - LO
RC = 384                 # table row stride in bf16 elements (768B)
WCOL = 288               # columns actually written (576B, 64B-aligned)
C = 260                  # used columns of a table row (xh + s_src)
RW = 264                 # matmul content cols: xh + s_src + s_dst
PAD_DST = 999.0


def preprocess(x, edge_index, W_lin, attn_src, attn_dst, W_out, b_out, ln_g, ln_b):
    """Returns (in_maps, (K_lo, K_hi, flags), scatter_info)."""
    x = np.asarray(x, np.float32)
    ei = np.asarray(edge_index)
    dst = ei[0].astype(np.int64)
    src = ei[1].astype(np.int64)
    W_lin = np.asarray(W_lin, np.float32)
    attn_src = np.asarray(attn_src, np.float32)
    attn_dst = np.asarray(attn_dst, np.float32)
    W_out = np.asarray(W_out, np.float32)
    b_out = np.asarray(b_out, np.float32)
    ln_g = np.asarray(ln_g, np.float32)
    ln_b = np.asarray(ln_b, np.float32)

    deg = np.bincount(dst, minlength=N)

    # --- pack nodes into WG windows: <=128 nodes each, balanced edge sums ---
    order = np.argsort(-deg, kind="stable")
    heap = [(0, w) for w in range(WG)]
    heapq.heapify(heap)
    win_nodes = [[] for _ in range(WG)]
    win_sum = [0] * WG
    for v in order:
        s, w = heapq.heappop(heap)
        win_nodes[w].append(v)
        win_sum[w] = s + int(deg[v])
        if len(win_nodes[w]) < 128:
            heapq.heappush(heap, (win_sum[w], w))

    slot_nodes = np.zeros((WG, 128), np.int64)
    slot_valid = np.zeros((WG, 128), bool)
    for w in range(WG):
        n = len(win_nodes[w])
        slot_nodes[w, :n] = win_nodes[w]
        slot_valid[w, :n] = True

    window_of = np.empty(N, np.int64)
    pos_in_window = np.empty(N, np.int64)
    window_of[slot_nodes[slot_valid]] = np.nonzero(slot_valid)[0]
    pos_in_window[slot_nodes[slot_valid]] = np.nonzero(slot_valid)[1]

    core_of_edge = window_of[dst] // WPC

    # consts shared by all cores
    ident = np.eye(128, dtype=np.float32).astype(NPBF)
    v_src = np.stack([W_lin[h * D:(h + 1) * D, :].T @ attn_src[h] for h in range(H)], axis=1)
    v_dst = np.stack([W_lin[h * D:(h + 1) * D, :].T @ attn_dst[h] for h in range(H)], axis=1)
    rhs = np.concatenate(
        [W_lin.T, v_src, v_dst, np.zeros((D, WCOL - RW), np.float32)], axis=1
    ).astype(NPBF)                                    # [64, 288]
    woutb = np.ascontiguousarray(W_out.T).astype(NPBF)  # [256, 64]
    bout_row = b_out.reshape(1, D).astype(NPBF)       # [1, 64]
    ones_row = np.ones((1, 128), NPBF)
    lng = np.tile(ln_g.reshape(1, 1, D), (128, 2, 1)).astype(np.float32).reshape(128, 2 * D)
    lnb = np.tile(ln_b.reshape(1, 1, D), (128, 2, 1)).astype(np.float32).reshape(128, 2 * D)

    # first pass per core: per-window lo/hi edge counts to size K_lo/K_hi
    per_core = []
    max_lo = max_hi = 0
    for c in range(NCORES):
        own = np.arange(c * WPC, (c + 1) * WPC)
        others = np.concatenate([np.arange(0, c * WPC), np.arange((c + 1) * WPC, WG)])
        worder = np.concatenate([own, others])
        perm = slot_nodes[worder].reshape(-1)
        val = slot_valid[worder].reshape(-1)
        row_of = np.empty(N, np.int64)
        row_of[perm[val]] = np.nonzero(val)[0]

        eidx = np.nonzero(core_of_edge == c)[0]
        wl = (window_of[dst[eidx]] - c * WPC).astype(np.int64)
        srow = row_of[src[eidx]]
        islo = srow < LO
        nlo = np.bincount(wl[islo], minlength=WPC)
        nhi = np.bincount(wl[~islo], minlength=WPC)
        max_lo = max(max_lo, int(nlo.max()))
        max_hi = max(max_hi, int(nhi.max()))
        per_core.append((perm, row_of, eidx, wl, srow, islo, nlo, nhi))

    K_lo = math.ceil(max_lo / 128)
    K_hi = math.ceil(max_hi / 128)
    K = K_lo + K_hi
    K2 = 2 * K
    cnt_lo = np.max(np.stack([pc[6] for pc in per_core]), axis=0)  # [WPC]
    cnt_hi = np.max(np.stack([pc[7] for pc in per_core]), axis=0)

    in_maps = []
    for c in range(NCORES):
        perm, row_of, eidx, wl, srow, islo, _, _ = per_core[c]
        xTp = np.ascontiguousarray(x[perm].T).astype(NPBF)  # [64, ROWS]
        xres = np.ascontiguousarray(
            (x[perm[:WPC * 128]] - 1.0).reshape(WPC, 128, D).transpose(1, 0, 2).reshape(128, WPC * D))

        # per-pair slot assignment; pair chunk layout:
        #   [w0-lo(K_lo) | w1-lo(K_lo) | w0-hi(K_hi) | w1-hi(K_hi)]
        idx_lo = np.zeros((NP, 2 * K_lo * 128), np.int16)
        idx_hi = np.zeros((NP, 2 * K_hi * 128), np.int16)
        dstloc = np.full((NP, 128, K2), PAD_DST, np.float32)
        for mask, base, ishi in [(islo, 0, 0), (~islo, LO, 1)]:
            sel = np.nonzero(mask)[0]
            wls = wl[sel]
            o2 = np.argsort(wls, kind="stable")
            sel = sel[o2]
            wls = wls[o2]
            counts = np.bincount(wls, minlength=WPC)
            starts = np.concatenate([[0], np.cumsum(counts)[:-1]])
            s = np.arange(len(sel)) - starts[wls]       # slot within window region
            pr = wls // 2                               # pair
            j = wls % 2                                 # window within pair
            Kr = K_hi if ishi else K_lo
            spos = j * Kr * 128 + s                     # position in region stream
            if ishi:
                idx_hi[pr, spos] = (srow[sel] - base).astype(np.int16)
            else:
                idx_lo[pr, spos] = (srow[sel] - base).astype(np.int16)
            ch = (2 * K_lo if ishi else 0) + j * Kr + s // 128  # pair chunk
            dstloc[pr, s % 128, ch] = pos_in_window[dst[eidx[sel]]].astype(np.float32)

        # wrap int16 indices: [128, L//16] (16-partition wrap replicated x8)
        idx16 = np.zeros((128, NP * K2 * 8), np.int16)
        for pr in range(NP):
            colbase = pr * K2 * 8
            blk_lo = idx_lo[pr].reshape(2 * K_lo * 8, 16).T
            idx16[:, colbase:colbase + 2 * K_lo * 8] = np.tile(blk_lo, (8, 1))
            if K_hi:
                blk_hi = idx_hi[pr].reshape(2 * K_hi * 8, 16).T
                idx16[:, colbase + 2 * K_lo * 8:colbase + K2 * 8] = np.tile(blk_hi, (8, 1))

        # one-hot matrices, fp8: st (slot-major) and snm (dst-major)
        r128 = np.arange(128, dtype=np.float32)
        st3 = (dstloc[:, :, :, None] == r128).astype(NPOH)       # [NP,128,K2,128]
        snm3 = np.ascontiguousarray(st3.transpose(0, 3, 2, 1))   # [NP,128,K2,128]
        oh = np.concatenate(
            [st3.reshape(NP, 128, K2 * 128), snm3.reshape(NP, 128, K2 * 128)], axis=2
        ).transpose(1, 0, 2).reshape(128, NP * 2 * K2 * 128)
        oh = np.ascontiguousarray(oh)

        in_maps.append({
            "xTp": xTp, "xres": xres, "idx16": idx16, "oh": oh,
            "ident": ident, "rhs": rhs, "woutb": woutb,
            "bout_row": bout_row, "ones_row": ones_row,
            "lng": lng, "lnb": lnb,
            "epsc": np.full((128, 1), 1e-5, np.float32),
        })

    flags = {
        "skip_bout": bool(np.all(b_out == 0.0)),
        "skip_ln_affine": bool(np.all(ln_g == 1.0) and np.all(ln_b == 0.0)),
        "cnt_lo": [int(v) for v in cnt_lo],
        "cnt_hi": [int(v) for v in cnt_hi],
    }
    scatter = (slot_nodes, slot_valid)
    return in_maps, (K_lo, K_hi, flags), scatter


def postprocess(results, scatter):
    slot_nodes, slot_valid = scatter
    y = np.empty((N, D), np.float32)
    for c in range(NCORES):
        oc = results[c]["y"]
        own = np.arange(c * WPC, (c + 1) * WPC)
        nodes = slot_nodes[own].reshape(-1)
        val = slot_valid[own].reshape(-1)
        y[nodes[val]] = oc[val]
    return y


def _filter_act_tables():
    """Keep only natural_log_exp_and_others as a loadable ACT set (indices
    preserved) so every activation in the kernel shares one table load."""
    import concourse.hw_specs as hw_specs
    if getattr(hw_specs, "_gat_patched", False):
        return
    orig = hw_specs.get_activation_tables

    def patched(module_arch):
        tabs = orig(module_arch)
        keep = "natural_log_exp_and_others"
        if keep in tabs:
            tabs = {k: (v if k == keep else set()) for k, v in tabs.items()}
        return tabs

    hw_specs.get_activation_tables = patched
    try:
        import concourse.bacc as _bacc_mod
        if getattr(_bacc_mod, "get_activation_tables", None) is orig:
            _bacc_mod.get_activation_tables = patched
    except Exception:
        pass
    hw_specs._gat_patched = True


def build_nc(K_lo, K_hi, flags=None, num_devices=NCORES, debug=False):
    flags = flags or {}
    _filter_act_tables()
    K = K_lo + K_hi
    K2 = 2 * K

    def chunk_owner(ch):
        return ch // K_lo if ch < 2 * K_lo else (ch - 2 * K_lo) // K_hi

    nc = bacc.Bacc("TRN2", target_bir_lowering=False, debug=False,
                   num_devices=num_devices, num_swdge_queues=4)
    xTp_d = nc.dram_tensor("xTp", [D, ROWS], BF16, kind="ExternalInput")
    xres_d = nc.dram_tensor("xres", [128, WPC * D], F32, kind="ExternalInput")
    idx16_d = nc.dram_tensor("idx16", [128, NP * K2 * 8], I16, kind="ExternalInput")
    oh_d = nc.dram_tensor("oh", [128, NP * 2 * K2 * 128], OH, kind="ExternalInput")
    ident_d = nc.dram_tensor("ident", [128, 128], BF16, kind="ExternalInput")
    rhs_d = nc.dram_tensor("rhs", [D, WCOL], BF16, kind="ExternalInput")
    woutb_d = nc.dram_tensor("woutb", [H * D, D], BF16, kind="ExternalInput")
    bout_d = nc.dram_tensor("bout_row", [1, D], BF16, kind="ExternalInput")
    ones_d = nc.dram_tensor("ones_row", [1, 128], BF16, kind="ExternalInput")
    lng_d = nc.dram_tensor("lng", [128, 2 * D], F32, kind="ExternalInput")
    lnb_d = nc.dram_tensor("lnb", [128, 2 * D], F32, kind="ExternalInput")
    epsc_d = nc.dram_tensor("epsc", [128, 1], F32, kind="ExternalInput")
    y_d = nc.dram_tensor("y", [WPC * 128, D], F32, kind="ExternalOutput")
    table_lo = nc.dram_tensor("table_lo", [LO, RC], BF16)
    table_hi = nc.dram_tensor("table_hi", [ROWS - LO, RC], BF16)

    with tile.TileContext(nc) as tc:
        with tc.tile_pool(name="const", bufs=1) as cp:
            ident = cp.tile([128, 128], BF16); nc.sync.dma_start(out=ident[:], in_=ident_d[:])
            rhs = cp.tile([D, WCOL], BF16); nc.sync.dma_start(out=rhs[:], in_=rhs_d[:])
            wout0 = cp.tile([128, D], BF16); nc.sync.dma_start(out=wout0[:], in_=woutb_d[0:128, :])
            wout1 = cp.tile([128, D], BF16); nc.sync.dma_start(out=wout1[:], in_=woutb_d[128:256, :])
            boutr = cp.tile([1, D], BF16); nc.sync.dma_start(out=boutr[:], in_=bout_d[:])
            onesr = cp.tile([1, 128], BF16); nc.sync.dma_start(out=onesr[:], in_=ones_d[:])
            lng = cp.tile([128, 2 * D], F32); nc.sync.dma_start(out=lng[:], in_=lng_d[:])
            lnb = cp.tile([128, 2 * D], F32); nc.sync.dma_start(out=lnb[:], in_=lnb_d[:])
            epsc = cp.tile([128, 1], F32); nc.sync.dma_start(out=epsc[:], in_=epsc_d[:])
            xres = cp.tile([128, WPC * D], F32); nc.sync.dma_start(out=xres[:], in_=xres_d[:])
            idx16 = cp.tile([128, NP * K2 * 8], I16); nc.sync.dma_start(out=idx16[:], in_=idx16_d[:])
            sdstb = cp.tile([128, WPC * H], BF16)

            # ---- stage 1: build table, 4 windows per PSUM tile/copy/DMA ----
            XCH = 16
            WB = 4
            with tc.tile_pool(name="s1x", bufs=3) as s1x, \
                 tc.tile_pool(name="s1row", bufs=3) as s1row, \
                 tc.tile_pool(name="s1p", bufs=2, space="PSUM") as s1p:
                for wb in range(0, WG, XCH):
                    xt = s1x.tile([D, XCH * 128], BF16, tag="xt")
                    nc.sync.dma_start(out=xt[:], in_=xTp_d[:, wb * 128:(wb + XCH) * 128])
                    for g4 in range(0, XCH, WB):
                        w0 = wb + g4
                        ps = s1p.tile([128, WB, 512], F32, tag="ps")
                        for j in range(WB):
                            nc.tensor.matmul(ps[:, j, 0:WCOL],
                                             lhsT=xt[:, (g4 + j) * 128:(g4 + j + 1) * 128],
                                             rhs=rhs[:], start=True, stop=True)
                        row4 = s1row.tile([128, WB, WCOL], BF16, tag="row")
                        if (w0 // WB) % 2 == 0:
                            nc.scalar.activation(row4[:], ps[:, :, 0:WCOL], ACT.Copy)
                        else:
                            nc.vector.tensor_copy(row4[:], ps[:, :, 0:WCOL])
                        n_own = max(0, min(WB, WPC - w0))
                        if n_own > 0:
                            nc.vector.tensor_copy(
                                sdstb[:, w0 * H:(w0 + n_own) * H].rearrange(
                                    "p (t f) -> p t f", f=H),
                                row4[:, 0:n_own, 260:264])
                        r0 = w0 * 128
                        dt_ = table_lo if r0 < LO else table_hi
                        base = r0 if r0 < LO else r0 - LO
                        nc.sync.dma_start(
                            out=dt_[base:base + WB * 128, 0:WCOL].rearrange(
                                "(t p) f -> p t f", p=128),
                            in_=row4[:])

            # ---- stage 2: per window-pair message passing ----
            with tc.tile_pool(name="gat", bufs=3) as gat, \
                 tc.tile_pool(name="ohp", bufs=4) as ohp, \
                 tc.tile_pool(name="mp", bufs=2) as mpp, \
                 tc.tile_pool(name="aop", bufs=2) as aop, \
                 tc.tile_pool(name="atp", bufs=2) as atp, \
                 tc.tile_pool(name="sm", bufs=8) as sm, \
                 tc.tile_pool(name="pSd", bufs=2, space="PSUM") as pSd, \
                 tc.tile_pool(name="pSeg", bufs=1, space="PSUM") as pSeg, \
                 tc.tile_pool(name="pT", bufs=2, space="PSUM") as pT, \
                 tc.tile_pool(name="pP", bufs=2, space="PSUM") as pP:

                g_t = [None] * NP
                oh_t = [None] * NP
                sd_t = [None] * NP
                aex_t = [None] * NP

                cnt_lo = flags.get("cnt_lo") or [K_lo * 128] * WPC
                cnt_hi = flags.get("cnt_hi") or [K_hi * 128] * WPC

                def prep(p):
                    # 4 gathers per pair (one per window per region): each must
                    # stay under the 1024-descriptor SWDGE ring carveout.
                    g = gat.tile([128, K2 * RC], BF16, tag="g")
                    off = p * K2 * 8
                    for j in range(2):
                        nc.gpsimd.dma_gather(
                            out_ap=g[:, j * K_lo * RC:(j + 1) * K_lo * RC]
                                .rearrange("p (k e) -> p k e", e=RC),
                            in_ap=table_lo[:],
                            idxs_ap=idx16[:, off + j * K_lo * 8:off + (j + 1) * K_lo * 8],
                            num_idxs=K_lo * 128, num_idxs_reg=K_lo * 128,
                            elem_size=RC, queue_num=(4 * p + j) % 4)
                        nc.gpsimd.dma_gather(
                            out_ap=g[:, (2 * K_lo + j * K_hi) * RC:
                                     (2 * K_lo + (j + 1) * K_hi) * RC]
                                .rearrange("p (k e) -> p k e", e=RC),
                            in_ap=table_hi[:],
                            idxs_ap=idx16[:, off + 2 * K_lo * 8 + j * K_hi * 8:
                                          off + 2 * K_lo * 8 + (j + 1) * K_hi * 8],
                            num_idxs=K_hi * 128, num_idxs_reg=K_hi * 128,
                            elem_size=RC, queue_num=(4 * p + 2 + j) % 4)
                    g_t[p] = g
                    oh = ohp.tile([128, 2 * K2 * 128], OH, tag="oh")
                    nc.scalar.dma_start(
                        out=oh[:], in_=oh_d[:, p * 2 * K2 * 128:(p + 1) * 2 * K2 * 128])
                    oh_t[p] = oh

                def alpha(p):
                    # s_dst per slot via dst-major one-hot matmuls, then
                    # alpha_exp into SBUF (written later into mval cols 256:260)
                    g, oh = g_t[p], oh_t[p]
                    g3 = g[:].rearrange("p (k f) -> p k f", f=RC)
                    sd = pSd.tile([128, K2 * H], F32, tag="sd")
                    for ch in range(K2):
                        nc.tensor.matmul(
                            sd[:, ch * H:(ch + 1) * H],
                            lhsT=oh[:, (K2 + ch) * 128:(K2 + ch + 1) * 128],
                            rhs=sdstb[:, (2 * p + chunk_owner(ch)) * H:
                                      (2 * p + chunk_owner(ch) + 1) * H],
                            start=True, stop=True)
                    apre = sm.tile([128, K2 * H], F32, tag="apre")
                    nc.vector.tensor_tensor(
                        out=apre[:].rearrange("p (k h) -> p k h", h=H),
                        in0=g3[:, :, 256:260],
                        in1=sd[:].rearrange("p (k h) -> p k h", h=H),
                        op=OP.add)
                    lr = sm.tile([128, K2 * H], F32, tag="lr")
                    nc.scalar.activation(lr[:], apre[:], ACT.Prelu, alpha=0.2)
                    aex = sm.tile([128, K2 * H], BF16, tag="aex")
                    nc.scalar.activation(aex[:], lr[:], ACT.Exp)
                    sd_t[p] = sd
                    aex_t[p] = aex

                def tail(p):
                    g, oh, aex = g_t[p], oh_t[p], aex_t[p]
                    g3 = g[:].rearrange("p (k f) -> p k f", f=RC)
                    a3 = aex[:].rearrange("p (k h) -> p k h", h=H)

                    # weighted messages [128, K2, 260]
                    mval = mpp.tile([128, K2 * C], BF16, tag="m")
                    m3 = mval[:].rearrange("p (k f) -> p k f", f=C)
                    nc.scalar.activation(m3[:, :, 256:260], a3, ACT.Copy)
                    nc.vector.tensor_tensor(
                        out=m3[:, :, 0:256].rearrange("p k (h d) -> p k h d", d=D),
                        in0=g3[:, :, 0:256].rearrange("p k (h d) -> p k h d", d=D),
                        in1=a3.unsqueeze(-1).to_broadcast([128, K2, H, D]),
                        op=OP.mult)

                    # segment matmuls: per window j over its chunks, skipping
                    # chunks that are entirely padding (their one-hot rows are
                    # all zero, so they contribute nothing)
                    seg = pSeg.tile([128, 2, 512], F32, tag="seg")
                    for j in range(2):
                        w = 2 * p + j
                        chunks = (
                            [j * K_lo + i for i in range(K_lo)
                             if i * 128 < max(1, cnt_lo[w])] +
                            [2 * K_lo + j * K_hi + i for i in range(K_hi)
                             if i * 128 < max(1, cnt_hi[w])])
                        for ki, ch in enumerate(chunks):
                            nc.tensor.matmul(
                                seg[:, j, 0:C],
                                lhsT=oh[:, ch * 128:(ch + 1) * 128],
                                rhs=mval[:, ch * C:(ch + 1) * C],
                                start=(ki == 0), stop=(ki == len(chunks) - 1))

                    # normalize by denominators -> ao bf16 [128, 2, 256]
                    d1 = sm.tile([128, 2 * H], F32, tag="d1")
                    nc.vector.tensor_scalar_add(
                        d1[:].rearrange("p (t h) -> p t h", h=H),
                        seg[:, :, 256:260], 1e-9)
                    rec = sm.tile([128, 2 * H], F32, tag="rec")
                    nc.vector.reciprocal(rec[:], d1[:])
                    ao = aop.tile([128, 2, 256], BF16, tag="ao")
                    nc.vector.tensor_tensor(
                        out=ao[:].rearrange("p t (h d) -> p t h d", d=D),
                        in0=seg[:, :, 0:256].rearrange("p t (h d) -> p t h d", d=D),
                        in1=rec[:].rearrange("p (t h) -> p t h", h=H)
                            .unsqueeze(-1).to_broadcast([128, 2, H, D]),
                        op=OP.mult)

                    # transpose ao -> aTs [128, 4, 128] (hd-major chunks)
                    tp = pT.tile([128, 4, 128], BF16, tag="tp")
                    aof = ao[:].rearrange("p t f -> p (t f)")
                    for i in range(4):
                        nc.tensor.transpose(tp[:, i, :], aof[:, i * 128:(i + 1) * 128],
                                            ident[:])
                    aTs = atp.tile([128, 4, 128], BF16, tag="aTs")
                    nc.scalar.activation(aTs[:], tp[:], ACT.Copy)

                    # project node-major: prj[node, d] = sum_hd aT[hd,node]*woutT[hd,d]
                    prj = pP.tile([128, 2, D], F32, tag="prj")
                    for j in range(2):
                        nc.tensor.matmul(prj[:, j, :], lhsT=aTs[:, 2 * j, :],
                                         rhs=wout0[:], start=True, stop=False)
                        nc.tensor.matmul(prj[:, j, :], lhsT=aTs[:, 2 * j + 1, :],
                                         rhs=wout1[:],
                                         start=False, stop=flags.get("skip_bout", False))
                        if not flags.get("skip_bout"):
                            nc.tensor.matmul(prj[:, j, :], lhsT=onesr[:],
                                             rhs=boutr[:], start=False, stop=True)

                    # ELU + residual(x-1): y2 = max(o,0) + exp(min(o,0)) + (x-1)
                    mn = sm.tile([128, 2 * D], F32, tag="mn")
                    nc.vector.tensor_scalar_min(
                        mn[:].rearrange("p (t f) -> p t f", f=D), prj[:], 0.0)
                    ex = sm.tile([128, 2 * D], F32, tag="ex")
                    nc.scalar.activation(ex[:], mn[:], ACT.Exp)
                    px = sm.tile([128, 2 * D], F32, tag="px")
                    nc.vector.tensor_scalar_max(
                        px[:].rearrange("p (t f) -> p t f", f=D), prj[:], 0.0)
                    y1 = sm.tile([128, 2 * D], F32, tag="y1")
                    nc.vector.tensor_tensor(out=y1[:], in0=px[:], in1=ex[:], op=OP.add)
                    y2 = sm.tile([128, 2 * D], F32, tag="y2")
                    nc.vector.tensor_tensor(
                        out=y2[:], in0=y1[:],
                        in1=xres[:, 2 * p * D:(2 * p + 2) * D], op=OP.add)

                    # LayerNorm via bn_stats; rstd = exp(-0.5 ln(var + eps))
                    y23 = y2[:].rearrange("p (t f) -> p t f", f=D)
                    stats = sm.tile([128, 2, 6], F32, tag="stats")
                    nc.vector.bn_stats(out=stats[:, 0, :], in_=y23[:, 0, :])
                    nc.vector.bn_stats(out=stats[:, 1, :], in_=y23[:, 1, :])
                    mv = sm.tile([128, 2, 2], F32, tag="mv")
                    nc.vector.bn_aggr(out=mv[:, 0, :], in_=stats[:, 0, :])
                    nc.vector.bn_aggr(out=mv[:, 1, :], in_=stats[:, 1, :])
                    lnv = sm.tile([128, 2], F32, tag="lnv")
                    nc.scalar.activation(lnv[:], mv[:, :, 1], ACT.Ln, bias=epsc[:, 0:1])
                    rstd = sm.tile([128, 2], F32, tag="rstd")
                    nc.scalar.activation(rstd[:], lnv[:], ACT.Exp, scale=-0.5)
                    cen = sm.tile([128, 2 * D], F32, tag="cen")
                    nc.vector.tensor_tensor(
                        out=cen[:].rearrange("p (t f) -> p t f", f=D),
                        in0=y23,
                        in1=mv[:, :, 0:1].to_broadcast([128, 2, D]),
                        op=OP.subtract)
                    f1 = sm.tile([128, 2 * D], F32, tag="f1")
                    nc.vector.tensor_tensor(
                        out=f1[:].rearrange("p (t f) -> p t f", f=D),
                        in0=cen[:].rearrange("p (t f) -> p t f", f=D),
                        in1=rstd[:].unsqueeze(-1).to_broadcast([128, 2, D]),
                        op=OP.mult)
                    if not flags.get("skip_ln_affine"):
                        f2 = sm.tile([128, 2 * D], F32, tag="f2")
                        nc.vector.tensor_tensor(out=f2[:], in0=f1[:], in1=lng[:], op=OP.mult)
                        f3 = sm.tile([128, 2 * D], F32, tag="f3")
                        nc.vector.tensor_tensor(out=f3[:], in0=f2[:], in1=lnb[:], op=OP.add)
                        f1 = f3
                    nc.sync.dma_start(
                        out=y_d[p * 256:(p + 1) * 256, :].rearrange("(t p) f -> p t f", p=128),
                        in_=f1[:].rearrange("p (t f) -> p t f", f=D))
                    g_t[p] = oh_t[p] = sd_t[p] = aex_t[p] = None

                for p0 in range(min(3, NP)):
                    prep(p0)
                alpha(0)
                for p in range(NP):
                    if p + 1 < NP:
                        alpha(p + 1)
                    tail(p)
                    if p + 3 < NP:
                        prep(p + 3)

    nc.finalize()
    return nc


def run(inputs, trace=False, num_devices=NCORES):
    in_maps, (K_lo, K_hi, flags), scatter = preprocess(**inputs)
    print("K_lo, K_hi, flags:", K_lo, K_hi, flags)
    nc = build_nc(K_lo, K_hi, flags, num_devices=num_devices)
    res = run_bass_kernel_spmd(nc, in_maps, core_ids=list(range(num_devices)), trace=trace)
    y = postprocess(res.results, scatter)
    return y, res


def kernel(**inputs):
    """Full-input MultiHeadGAT layer on 8 TRN2 NeuronCores."""
    y, _ = run(inputs, trace=False)
    return y
